# revision 62
# baseline (speedup 1.0000x reference)
"""Additive (Bahdanau) attention energy kernel for 8 TRN2 NeuronCores.

energy[b,h,q,k] = sum_d V_w[d] * tanh( (Q@W1^T+b1)[q,d] + (K@W2^T+b2)[k,d] ) + V_b

Sharding: the 16 (b,h) pairs are split 2-per-core (data parallel); the tiny
W1/W2/V weights are replicated.  Per core, both (b,h) pairs are processed
simultaneously by packing partitions as (d in 0..63, bh in 0..1).

Per-core device pipeline (q = 0..511 outer loop):
  DVE : stage[:, j*512:(j+1)*512] = k2T_stack + q1T_stack[:, q]  (bf16,
        packed 2x perf mode; q1 column is the per-partition scalar operand)
  ACT : one Tanh instruction per batch of up to 16 q's ([128, 8192]) —
        this is the bottleneck engine, saturated at ~(FD+224)cyc/1.2GHz
  PE  : energy rows via M=32 accumulating bf16 matmuls at PSUM base 0:
        lhsT variant t holds V_w at cols (2t, 2t+1) (rows 0..63 / 64..127),
        zeros elsewhere -> 16 matmuls fill a [32,512] PSUM tile densely
        (nonzero matmul dst partitions are rejected for f32r on this
        toolchain, and M=2 writes would leave PSUM banks 94% empty).
  DVE : evict [32,512] PSUM tile (16 q's x 2 bh) + add V_b
  DMA : staging -> DRAM out (rows interleaved r = 2q + bh)

Measured on HW: ~248 us NEFF exec (all 8 cores within 1 us), rel err 2.4e-3.
ACT busy is ~228.5 us with zero idle — the tanh-engine roofline for this
shard size is 33.5M elems / (128 lanes * 1.2 GHz) = 218.5 us.
"""

import numpy as np

import concourse.bass as bass
import concourse.mybir as mybir
from concourse import bacc
from concourse.tile import TileContext

F32 = mybir.dt.float32
F32R = mybir.dt.float32r

B, H, SQ, SK, D = 2, 8, 512, 512, 64
NCORES = 8
BH = B * H            # 16 flat (b,h) pairs
BH_PER_CORE = BH // NCORES  # 2
QPT = 16              # q's per PSUM tile (16 matmuls x 2 rows)
# activation batch sizes: a small head batch starts the ACT stream early,
# a small tail batch lets the PE/eviction tail drain early; 16-q batches
# amortize the per-instruction ACT overhead (224 cyc each) while keeping
# 3-deep buffering so ACT never waits on DVE/PE
# graduated ramp: DVE (345 ns/q) must stay ahead of ACT (~434 ns/q) with
# only 3 stage slots, so batch size may not jump too fast
BATCHES = [2, 2, 4, 8, 16, 16] + [24] * 18 + [16, 8, 4, 2, 2]
MAXB = max(BATCHES)

# matmul input mode: "f32r" (near-fp32, but streams at half rate / stays
# HAM-cold on this silicon) or "bf16" (full-rate, FWL weight loads)
MM_MODE = "bf16"


def build_nc():
    nc = bacc.Bacc("TRN2", target_bir_lowering=False)

    # Inputs are pre-marshalled on the host:
    #   QTS/KTS: [128, 512] transposed activations, both bh stacked on rows
    #   W1TB/W2TB: [128, 128] block-diagonal transposed projection weights
    # All four are bf16: halves the head DMA and runs the projections at
    # full matmul rate (precision cost ~1e-3 rel, gate is 2e-2).
    BF16 = mybir.dt.bfloat16
    QTS = nc.dram_tensor("QTS", [128, SQ], BF16, kind="ExternalInput")
    KTS = nc.dram_tensor("KTS", [128, SK], BF16, kind="ExternalInput")
    W1TB = nc.dram_tensor("W1TB", [128, 128], BF16, kind="ExternalInput")
    W2TB = nc.dram_tensor("W2TB", [128, 128], BF16, kind="ExternalInput")
    # CONSTS columns: 0 = W1_b stacked x2, 1 = W2_b stacked x2, 2 = V_b bcast
    CONSTS = nc.dram_tensor("CONSTS", [128, 3], F32, kind="ExternalInput")
    # VVARS: 16 lhsT variants side by side; variant t has V_w at col 2t
    # (rows 0..63) and col 2t+1 (rows 64..127), zeros elsewhere
    VVARS = nc.dram_tensor("VVARS", [128, 512], F32, kind="ExternalInput")
    # rows interleaved as (q, bh): row 2*q+bh — de-interleaved on the host
    out = nc.dram_tensor("out", [SQ * BH_PER_CORE, SK], F32, kind="ExternalOutput")

    mm_dt = F32R if MM_MODE == "f32r" else mybir.dt.bfloat16

    with TileContext(nc) as tc:
        with (
            tc.tile_pool(name="const", bufs=1) as cpool,
            tc.tile_pool(name="stage", bufs=3) as spool,
            tc.tile_pool(name="tanh", bufs=3) as tpool,
            tc.tile_pool(name="energy", bufs=3) as epool,
            tc.tile_pool(name="psum_e", bufs=3, space="PSUM") as ppool,
            tc.tile_pool(name="psum_s", bufs=2, space="PSUM") as pspool,
        ):
            # ---------------- setup ----------------
            # dependency-free warm-up activation: forces the ACT table load
            # (~1.3us) to run at the head, overlapped with the input DMAs,
            # instead of serializing before the first real activation
            warm = cpool.tile([1, 8], F32, tag="warm")
            nc.gpsimd.memset(warm[0:1, :], 0.0)
            nc.scalar.activation(
                warm[0:1, :], warm[0:1, :], mybir.ActivationFunctionType.Tanh
            )

            qts = cpool.tile([128, SQ], BF16, tag="qts")
            kts = cpool.tile([128, SK], BF16, tag="kts")
            w1tb = cpool.tile([128, 128], BF16, tag="w1tb")
            w2tb = cpool.tile([128, 128], BF16, tag="w2tb")
            nc.sync.dma_start(qts[:], QTS[:, :])
            nc.sync.dma_start(w1tb[:], W1TB[:, :])
            nc.sync.dma_start(kts[:], KTS[:, :])
            nc.sync.dma_start(w2tb[:], W2TB[:, :])

            consts = cpool.tile([128, 3], F32, tag="consts")
            nc.sync.dma_start(consts[:], CONSTS[:, :])
            w1b_st = consts[:, 0:1]
            w2b_st = consts[:, 1:2]
            vb_bc = consts[:, 2:3]

            vv_f32 = cpool.tile([128, 512], F32, tag="vv_f32")
            nc.sync.dma_start(vv_f32[:], VVARS[:, :])

            # projections: q1T_stack / k2T_stack  [128=(d,bh), 512]
            # k2t is bf16 so the per-q broadcast-add runs in the DVE packed
            # (2x) perf mode; q1t stays f32 (scalar operand is exempt).
            q1t = cpool.tile([128, SQ], F32, tag="q1t")
            k2t = cpool.tile([128, SK], mybir.dt.bfloat16, tag="k2t")
            ps_q1 = pspool.tile([128, SQ], F32, tag="setup")
            nc.tensor.matmul(ps_q1[:], w1tb[:], qts[:], start=True, stop=True)
            nc.vector.tensor_scalar_add(q1t[:], ps_q1[:], w1b_st)
            ps_k2 = pspool.tile([128, SK], F32, tag="setup")
            nc.tensor.matmul(ps_k2[:], w2tb[:], kts[:], start=True, stop=True)
            # k2 bias-add on the (head-idle) ACT engine, off the DVE chain
            # that feeds the first stage adds; identity shares tanh's table
            nc.scalar.activation(
                k2t[:], ps_k2[:], mybir.ActivationFunctionType.Identity,
                bias=w2b_st,
            )

            vvars = cpool.tile([128, 512], mm_dt, tag="vvars")

            # ---------------- main loop ----------------
            qcur = 0
            ps_e = None
            vvars_done = False
            for bsz in BATCHES:
                stage = spool.tile(
                    [128, MAXB * SK], mybir.dt.bfloat16, tag="stage"
                )
                for j in range(bsz):
                    q = qcur + j
                    nc.vector.tensor_scalar_add(
                        stage[:, j * SK : (j + 1) * SK],
                        k2t[:],
                        q1t[:, q : q + 1],
                    )
                if not vvars_done:
                    # round the host-built V variants to the matmul dtype;
                    # emitted after the first adds so the DVE reaches the
                    # first stage batch sooner (only the first matmul needs it)
                    nc.vector.tensor_copy(vvars[:], vv_f32[:])
                    vvars_done = True
                th = tpool.tile([128, MAXB * SK], mm_dt, tag="tanh")
                nc.scalar.activation(
                    th[:, 0 : bsz * SK],
                    stage[:, 0 : bsz * SK],
                    mybir.ActivationFunctionType.Tanh,
                )
                for j in range(bsz):
                    q = qcur + j
                    t = q % QPT  # variant within psum tile, 0..15
                    if t == 0:
                        ps_e = ppool.tile([32, 512], F32, tag="ps_e")
                    nc.tensor.matmul(
                        ps_e[:],
                        vvars[:, 32 * t : 32 * t + 32],
                        th[:, j * SK : (j + 1) * SK],
                        start=(t == 0),
                        stop=(t == QPT - 1),
                    )
                    if t == QPT - 1:
                        # rows p = 2*(q%16) + bh -> out row 32g + p = 2q + bh
                        g = q // QPT
                        ev = epool.tile([32, 512], F32, tag="ev")
                        nc.vector.tensor_scalar_add(
                            ev[:], ps_e[:], vb_bc[0:32, 0:1]
                        )
                        nc.sync.dma_start(out[32 * g : 32 * g + 32, :], ev[:])
                qcur += bsz

    nc.compile()
    return nc


_NC_CACHE = None
LAST_RESULTS = None


def _get_nc():
    global _NC_CACHE
    if _NC_CACHE is None:
        _NC_CACHE = build_nc()
    return _NC_CACHE


def make_in_maps(Q, K, W1_w, W1_b, W2_w, W2_b, V_w, V_b):
    """Host-side marshalling: shard (b,h) across cores, transpose layouts,
    prebuild the block-diagonal weights, constant columns and V variants."""
    import ml_dtypes

    f = np.float32
    bf = ml_dtypes.bfloat16
    Qf = np.ascontiguousarray(Q, dtype=f).reshape(BH, SQ, D)
    Kf = np.ascontiguousarray(K, dtype=f).reshape(BH, SK, D)
    W1T = np.asarray(W1_w, dtype=f).T  # [d, e]
    W2T = np.asarray(W2_w, dtype=f).T

    w1tb = np.zeros((128, 128), dtype=bf)
    w1tb[0:64, 0:64] = W1T.astype(bf)
    w1tb[64:128, 64:128] = W1T.astype(bf)
    w2tb = np.zeros((128, 128), dtype=bf)
    w2tb[0:64, 0:64] = W2T.astype(bf)
    w2tb[64:128, 64:128] = W2T.astype(bf)

    consts = np.zeros((128, 3), dtype=f)
    consts[0:64, 0] = consts[64:128, 0] = np.asarray(W1_b, dtype=f).ravel()
    consts[0:64, 1] = consts[64:128, 1] = np.asarray(W2_b, dtype=f).ravel()
    consts[:, 2] = np.asarray(V_b, dtype=f).ravel()[0]

    vvars = np.zeros((128, 512), dtype=f)
    vw = np.asarray(V_w, dtype=f).ravel()  # [64]
    for t in range(16):
        vvars[0:64, 32 * t + 2 * t] = vw
        vvars[64:128, 32 * t + 2 * t + 1] = vw

    in_maps = []
    for c in range(NCORES):
        sl = slice(c * BH_PER_CORE, (c + 1) * BH_PER_CORE)
        qts = np.ascontiguousarray(
            Qf[sl].transpose(0, 2, 1).reshape(128, SQ).astype(bf)
        )  # [2*64, SQ]
        kts = np.ascontiguousarray(
            Kf[sl].transpose(0, 2, 1).reshape(128, SK).astype(bf)
        )
        in_maps.append(
            {
                "QTS": qts,
                "KTS": kts,
                "W1TB": w1tb,
                "W2TB": w2tb,
                "CONSTS": consts,
                "VVARS": vvars,
            }
        )
    return in_maps


def kernel(**inputs) -> np.ndarray:
    global LAST_RESULTS
    from concourse.bass_utils import run_bass_kernel_spmd

    nc = _get_nc()
    in_maps = make_in_maps(**inputs)
    try:
        res = run_bass_kernel_spmd(nc, in_maps, core_ids=list(range(NCORES)))
    except Exception:
        # transient NRT device errors have been observed; retry once
        res = run_bass_kernel_spmd(nc, in_maps, core_ids=list(range(NCORES)))
    LAST_RESULTS = res
    per_core = [
        r["out"].reshape(SQ, BH_PER_CORE, SK).transpose(1, 0, 2) for r in res.results
    ]
    full = np.concatenate(per_core, axis=0)  # [16, 512, 512]
    return np.ascontiguousarray(full.reshape(B, H, SQ, SK), dtype=np.float32)


# revision 64
# speedup vs baseline: 1.0001x; 1.0001x over previous
"""Additive (Bahdanau) attention energy kernel for 8 TRN2 NeuronCores.

energy[b,h,q,k] = sum_d V_w[d] * tanh( (Q@W1^T+b1)[q,d] + (K@W2^T+b2)[k,d] ) + V_b

Sharding: the 16 (b,h) pairs are split 2-per-core (data parallel); the tiny
W1/W2/V weights are replicated.  Per core, both (b,h) pairs are processed
simultaneously by packing partitions as (d in 0..63, bh in 0..1).

Per-core device pipeline (q = 0..511 outer loop):
  DVE : stage[:, j*512:(j+1)*512] = k2T_stack + q1T_stack[:, q]  (bf16,
        packed 2x perf mode; q1 column is the per-partition scalar operand)
  ACT : one Tanh instruction per batch of up to 16 q's ([128, 8192]) —
        this is the bottleneck engine, saturated at ~(FD+224)cyc/1.2GHz
  PE  : energy rows via M=32 accumulating bf16 matmuls at PSUM base 0:
        lhsT variant t holds V_w at cols (2t, 2t+1) (rows 0..63 / 64..127),
        zeros elsewhere -> 16 matmuls fill a [32,512] PSUM tile densely
        (nonzero matmul dst partitions are rejected for f32r on this
        toolchain, and M=2 writes would leave PSUM banks 94% empty).
  DVE : evict [32,512] PSUM tile (16 q's x 2 bh) + add V_b
  DMA : staging -> DRAM out (rows interleaved r = 2q + bh)

Measured on HW: ~248 us NEFF exec (all 8 cores within 1 us), rel err 2.4e-3.
ACT busy is ~228.5 us with zero idle — the tanh-engine roofline for this
shard size is 33.5M elems / (128 lanes * 1.2 GHz) = 218.5 us.
"""

import numpy as np

import concourse.bass as bass
import concourse.mybir as mybir
from concourse import bacc
from concourse.tile import TileContext

F32 = mybir.dt.float32
F32R = mybir.dt.float32r

B, H, SQ, SK, D = 2, 8, 512, 512, 64
NCORES = 8
BH = B * H            # 16 flat (b,h) pairs
BH_PER_CORE = BH // NCORES  # 2
QPT = 16              # q's per PSUM tile (16 matmuls x 2 rows)
# activation batch sizes: a small head batch starts the ACT stream early,
# a small tail batch lets the PE/eviction tail drain early; 16-q batches
# amortize the per-instruction ACT overhead (224 cyc each) while keeping
# 3-deep buffering so ACT never waits on DVE/PE
# graduated ramp: DVE (345 ns/q) must stay ahead of ACT (~434 ns/q) with
# only 3 stage slots, so batch size may not jump too fast
BATCHES = [2, 2, 4, 8, 16, 16] + [24] * 18 + [16, 8, 4, 2, 2]
MAXB = max(BATCHES)

# matmul input mode: "f32r" (near-fp32, but streams at half rate / stays
# HAM-cold on this silicon) or "bf16" (full-rate, FWL weight loads)
MM_MODE = "bf16"


def build_nc():
    nc = bacc.Bacc("TRN2", target_bir_lowering=False)

    # Inputs are pre-marshalled on the host:
    #   QTS/KTS: [128, 512] transposed activations, both bh stacked on rows
    #   W1TB/W2TB: [128, 128] block-diagonal transposed projection weights
    # All four are bf16: halves the head DMA and runs the projections at
    # full matmul rate (precision cost ~1e-3 rel, gate is 2e-2).
    BF16 = mybir.dt.bfloat16
    QTS = nc.dram_tensor("QTS", [128, SQ], BF16, kind="ExternalInput")
    KTS = nc.dram_tensor("KTS", [128, SK], BF16, kind="ExternalInput")
    W1TB = nc.dram_tensor("W1TB", [128, 128], BF16, kind="ExternalInput")
    W2TB = nc.dram_tensor("W2TB", [128, 128], BF16, kind="ExternalInput")
    # CONSTS columns: 0 = W1_b stacked x2, 1 = W2_b stacked x2, 2 = V_b bcast
    CONSTS = nc.dram_tensor("CONSTS", [128, 3], F32, kind="ExternalInput")
    # VVARS: 16 lhsT variants side by side; variant t has V_w at col 2t
    # (rows 0..63) and col 2t+1 (rows 64..127), zeros elsewhere
    VVARS = nc.dram_tensor("VVARS", [128, 512], F32, kind="ExternalInput")
    # rows interleaved as (q, bh): row 2*q+bh — de-interleaved on the host
    out = nc.dram_tensor("out", [SQ * BH_PER_CORE, SK], F32, kind="ExternalOutput")

    mm_dt = F32R if MM_MODE == "f32r" else mybir.dt.bfloat16

    with TileContext(nc) as tc:
        with (
            tc.tile_pool(name="const", bufs=1) as cpool,
            tc.tile_pool(name="stage", bufs=3) as spool,
            tc.tile_pool(name="tanh", bufs=3) as tpool,
            tc.tile_pool(name="energy", bufs=3) as epool,
            tc.tile_pool(name="psum_e", bufs=3, space="PSUM") as ppool,
            tc.tile_pool(name="psum_s", bufs=2, space="PSUM") as pspool,
        ):
            # ---------------- setup ----------------
            # dependency-free warm-up activation: forces the ACT table load
            # (~1.3us) to run at the head, overlapped with the input DMAs,
            # instead of serializing before the first real activation
            warm = cpool.tile([1, 8], F32, tag="warm")
            nc.gpsimd.memset(warm[0:1, :], 0.0)
            nc.scalar.activation(
                warm[0:1, :], warm[0:1, :], mybir.ActivationFunctionType.Tanh
            )

            qts = cpool.tile([128, SQ], BF16, tag="qts")
            kts = cpool.tile([128, SK], BF16, tag="kts")
            w1tb = cpool.tile([128, 128], BF16, tag="w1tb")
            w2tb = cpool.tile([128, 128], BF16, tag="w2tb")
            nc.sync.dma_start(kts[:], KTS[:, :])
            nc.sync.dma_start(w2tb[:], W2TB[:, :])
            nc.sync.dma_start(qts[:], QTS[:, :])
            nc.sync.dma_start(w1tb[:], W1TB[:, :])

            consts = cpool.tile([128, 3], F32, tag="consts")
            nc.sync.dma_start(consts[:], CONSTS[:, :])
            w1b_st = consts[:, 0:1]
            w2b_st = consts[:, 1:2]
            vb_bc = consts[:, 2:3]

            vv_f32 = cpool.tile([128, 512], F32, tag="vv_f32")
            nc.sync.dma_start(vv_f32[:], VVARS[:, :])

            # projections: q1T_stack / k2T_stack  [128=(d,bh), 512]
            # k2t is bf16 so the per-q broadcast-add runs in the DVE packed
            # (2x) perf mode; q1t stays f32 (scalar operand is exempt).
            q1t = cpool.tile([128, SQ], F32, tag="q1t")
            k2t = cpool.tile([128, SK], mybir.dt.bfloat16, tag="k2t")
            ps_k2 = pspool.tile([128, SK], F32, tag="setup")
            nc.tensor.matmul(ps_k2[:], w2tb[:], kts[:], start=True, stop=True)
            # k2 bias-add on the (head-idle) ACT engine, off the DVE chain
            # that feeds the first stage adds; identity shares tanh's table
            nc.scalar.activation(
                k2t[:], ps_k2[:], mybir.ActivationFunctionType.Identity,
                bias=w2b_st,
            )
            ps_q1 = pspool.tile([128, SQ], F32, tag="setup")
            nc.tensor.matmul(ps_q1[:], w1tb[:], qts[:], start=True, stop=True)
            nc.vector.tensor_scalar_add(q1t[:], ps_q1[:], w1b_st)

            vvars = cpool.tile([128, 512], mm_dt, tag="vvars")

            # ---------------- main loop ----------------
            qcur = 0
            ps_e = None
            vvars_done = False
            for bsz in BATCHES:
                stage = spool.tile(
                    [128, MAXB * SK], mybir.dt.bfloat16, tag="stage"
                )
                for j in range(bsz):
                    q = qcur + j
                    nc.vector.tensor_scalar_add(
                        stage[:, j * SK : (j + 1) * SK],
                        k2t[:],
                        q1t[:, q : q + 1],
                    )
                if not vvars_done:
                    # round the host-built V variants to the matmul dtype;
                    # emitted after the first adds so the DVE reaches the
                    # first stage batch sooner (only the first matmul needs it)
                    nc.vector.tensor_copy(vvars[:], vv_f32[:])
                    vvars_done = True
                th = tpool.tile([128, MAXB * SK], mm_dt, tag="tanh")
                nc.scalar.activation(
                    th[:, 0 : bsz * SK],
                    stage[:, 0 : bsz * SK],
                    mybir.ActivationFunctionType.Tanh,
                )
                for j in range(bsz):
                    q = qcur + j
                    t = q % QPT  # variant within psum tile, 0..15
                    if t == 0:
                        ps_e = ppool.tile([32, 512], F32, tag="ps_e")
                    nc.tensor.matmul(
                        ps_e[:],
                        vvars[:, 32 * t : 32 * t + 32],
                        th[:, j * SK : (j + 1) * SK],
                        start=(t == 0),
                        stop=(t == QPT - 1),
                    )
                    if t == QPT - 1:
                        # rows p = 2*(q%16) + bh -> out row 32g + p = 2q + bh
                        g = q // QPT
                        ev = epool.tile([32, 512], F32, tag="ev")
                        nc.vector.tensor_scalar_add(
                            ev[:], ps_e[:], vb_bc[0:32, 0:1]
                        )
                        nc.sync.dma_start(out[32 * g : 32 * g + 32, :], ev[:])
                qcur += bsz

    nc.compile()
    return nc


_NC_CACHE = None
LAST_RESULTS = None


def _get_nc():
    global _NC_CACHE
    if _NC_CACHE is None:
        _NC_CACHE = build_nc()
    return _NC_CACHE


def make_in_maps(Q, K, W1_w, W1_b, W2_w, W2_b, V_w, V_b):
    """Host-side marshalling: shard (b,h) across cores, transpose layouts,
    prebuild the block-diagonal weights, constant columns and V variants."""
    import ml_dtypes

    f = np.float32
    bf = ml_dtypes.bfloat16
    Qf = np.ascontiguousarray(Q, dtype=f).reshape(BH, SQ, D)
    Kf = np.ascontiguousarray(K, dtype=f).reshape(BH, SK, D)
    W1T = np.asarray(W1_w, dtype=f).T  # [d, e]
    W2T = np.asarray(W2_w, dtype=f).T

    w1tb = np.zeros((128, 128), dtype=bf)
    w1tb[0:64, 0:64] = W1T.astype(bf)
    w1tb[64:128, 64:128] = W1T.astype(bf)
    w2tb = np.zeros((128, 128), dtype=bf)
    w2tb[0:64, 0:64] = W2T.astype(bf)
    w2tb[64:128, 64:128] = W2T.astype(bf)

    consts = np.zeros((128, 3), dtype=f)
    consts[0:64, 0] = consts[64:128, 0] = np.asarray(W1_b, dtype=f).ravel()
    consts[0:64, 1] = consts[64:128, 1] = np.asarray(W2_b, dtype=f).ravel()
    consts[:, 2] = np.asarray(V_b, dtype=f).ravel()[0]

    vvars = np.zeros((128, 512), dtype=f)
    vw = np.asarray(V_w, dtype=f).ravel()  # [64]
    for t in range(16):
        vvars[0:64, 32 * t + 2 * t] = vw
        vvars[64:128, 32 * t + 2 * t + 1] = vw

    in_maps = []
    for c in range(NCORES):
        sl = slice(c * BH_PER_CORE, (c + 1) * BH_PER_CORE)
        qts = np.ascontiguousarray(
            Qf[sl].transpose(0, 2, 1).reshape(128, SQ).astype(bf)
        )  # [2*64, SQ]
        kts = np.ascontiguousarray(
            Kf[sl].transpose(0, 2, 1).reshape(128, SK).astype(bf)
        )
        in_maps.append(
            {
                "QTS": qts,
                "KTS": kts,
                "W1TB": w1tb,
                "W2TB": w2tb,
                "CONSTS": consts,
                "VVARS": vvars,
            }
        )
    return in_maps


def kernel(**inputs) -> np.ndarray:
    global LAST_RESULTS
    from concourse.bass_utils import run_bass_kernel_spmd

    nc = _get_nc()
    in_maps = make_in_maps(**inputs)
    try:
        res = run_bass_kernel_spmd(nc, in_maps, core_ids=list(range(NCORES)))
    except Exception:
        # transient NRT device errors have been observed; retry once
        res = run_bass_kernel_spmd(nc, in_maps, core_ids=list(range(NCORES)))
    LAST_RESULTS = res
    per_core = [
        r["out"].reshape(SQ, BH_PER_CORE, SK).transpose(1, 0, 2) for r in res.results
    ]
    full = np.concatenate(per_core, axis=0)  # [16, 512, 512]
    return np.ascontiguousarray(full.reshape(B, H, SQ, SK), dtype=np.float32)


# revision 66
# speedup vs baseline: 1.0002x; 1.0001x over previous
"""Additive (Bahdanau) attention energy kernel for 8 TRN2 NeuronCores.

energy[b,h,q,k] = sum_d V_w[d] * tanh( (Q@W1^T+b1)[q,d] + (K@W2^T+b2)[k,d] ) + V_b

Sharding: the 16 (b,h) pairs are split 2-per-core (data parallel); the tiny
W1/W2/V weights are replicated.  Per core, both (b,h) pairs are processed
simultaneously by packing partitions as (d in 0..63, bh in 0..1).

Per-core device pipeline (q = 0..511 outer loop):
  DVE : stage[:, j*512:(j+1)*512] = k2T_stack + q1T_stack[:, q]  (bf16,
        packed 2x perf mode; q1 column is the per-partition scalar operand)
  ACT : one Tanh instruction per batch of up to 16 q's ([128, 8192]) —
        this is the bottleneck engine, saturated at ~(FD+224)cyc/1.2GHz
  PE  : energy rows via M=32 accumulating bf16 matmuls at PSUM base 0:
        lhsT variant t holds V_w at cols (2t, 2t+1) (rows 0..63 / 64..127),
        zeros elsewhere -> 16 matmuls fill a [32,512] PSUM tile densely
        (nonzero matmul dst partitions are rejected for f32r on this
        toolchain, and M=2 writes would leave PSUM banks 94% empty).
  DVE : evict [32,512] PSUM tile (16 q's x 2 bh) + add V_b
  DMA : staging -> DRAM out (rows interleaved r = 2q + bh)

Measured on HW: ~244.2-244.5 us NEFF exec in the chip's normal power state
(all 8 cores within 1 us), rel err 3.26e-3.  The chip intermittently enters
a ~20%-downclocked power/thermal state where the same NEFF reads ~291 us.
ACT busy is ~226.6 us with 245 ns total idle — the tanh-engine roofline for
this shard size is 33.5M elems / (128 lanes * 1.2 GHz) = 218.5 us; the rest
of the span is NEFF boot (~6 us), input DMA + projections (~6 us), and the
Tile end-of-kernel drain barrier (~9.5 us).
"""

import numpy as np

import concourse.bass as bass
import concourse.mybir as mybir
from concourse import bacc
from concourse.tile import TileContext

F32 = mybir.dt.float32
F32R = mybir.dt.float32r

B, H, SQ, SK, D = 2, 8, 512, 512, 64
NCORES = 8
BH = B * H            # 16 flat (b,h) pairs
BH_PER_CORE = BH // NCORES  # 2
QPT = 16              # q's per PSUM tile (16 matmuls x 2 rows)
# activation batch sizes: a small head batch starts the ACT stream early,
# a small tail batch lets the PE/eviction tail drain early; 16-q batches
# amortize the per-instruction ACT overhead (224 cyc each) while keeping
# 3-deep buffering so ACT never waits on DVE/PE
# graduated ramp: DVE (345 ns/q) must stay ahead of ACT (~434 ns/q) with
# only 3 stage slots, so batch size may not jump too fast
BATCHES = [2, 2, 4, 8, 16, 16] + [24] * 18 + [16, 8, 4, 2, 2]
MAXB = max(BATCHES)

# matmul input mode: "f32r" (near-fp32, but streams at half rate / stays
# HAM-cold on this silicon) or "bf16" (full-rate, FWL weight loads)
MM_MODE = "bf16"


def build_nc():
    nc = bacc.Bacc("TRN2", target_bir_lowering=False)

    # Inputs are pre-marshalled on the host:
    #   QTS/KTS: [128, 512] transposed activations, both bh stacked on rows
    #   W1TB/W2TB: [128, 128] block-diagonal transposed projection weights
    # All four are bf16: halves the head DMA and runs the projections at
    # full matmul rate (precision cost ~1e-3 rel, gate is 2e-2).
    BF16 = mybir.dt.bfloat16
    QTS = nc.dram_tensor("QTS", [128, SQ], BF16, kind="ExternalInput")
    KTS = nc.dram_tensor("KTS", [128, SK], BF16, kind="ExternalInput")
    W1TB = nc.dram_tensor("W1TB", [128, 128], BF16, kind="ExternalInput")
    W2TB = nc.dram_tensor("W2TB", [128, 128], BF16, kind="ExternalInput")
    # CONSTS columns: 0 = W1_b stacked x2, 1 = W2_b stacked x2, 2 = V_b bcast
    CONSTS = nc.dram_tensor("CONSTS", [128, 3], F32, kind="ExternalInput")
    # VVARS: 16 lhsT variants side by side; variant t has V_w at col 2t
    # (rows 0..63) and col 2t+1 (rows 64..127), zeros elsewhere
    VVARS = nc.dram_tensor("VVARS", [128, 512], F32, kind="ExternalInput")
    # rows interleaved as (q, bh): row 2*q+bh — de-interleaved on the host
    out = nc.dram_tensor("out", [SQ * BH_PER_CORE, SK], F32, kind="ExternalOutput")

    mm_dt = F32R if MM_MODE == "f32r" else mybir.dt.bfloat16

    with TileContext(nc) as tc:
        with (
            tc.tile_pool(name="const", bufs=1) as cpool,
            tc.tile_pool(name="stage", bufs=3) as spool,
            tc.tile_pool(name="tanh", bufs=3) as tpool,
            tc.tile_pool(name="energy", bufs=3) as epool,
            tc.tile_pool(name="psum_e", bufs=3, space="PSUM") as ppool,
            tc.tile_pool(name="psum_s", bufs=2, space="PSUM") as pspool,
        ):
            # ---------------- setup ----------------
            # dependency-free warm-up activation: forces the ACT table load
            # (~1.3us) to run at the head, overlapped with the input DMAs,
            # instead of serializing before the first real activation
            warm = cpool.tile([1, 8], F32, tag="warm")
            nc.gpsimd.memset(warm[0:1, :], 0.0)
            nc.scalar.activation(
                warm[0:1, :], warm[0:1, :], mybir.ActivationFunctionType.Tanh
            )

            qts = cpool.tile([128, SQ], BF16, tag="qts")
            kts = cpool.tile([128, SK], BF16, tag="kts")
            w1tb = cpool.tile([128, 128], BF16, tag="w1tb")
            w2tb = cpool.tile([128, 128], BF16, tag="w2tb")
            nc.sync.dma_start(qts[:], QTS[:, :])
            nc.sync.dma_start(w1tb[:], W1TB[:, :])
            nc.sync.dma_start(kts[:], KTS[:, :])
            nc.sync.dma_start(w2tb[:], W2TB[:, :])

            consts = cpool.tile([128, 3], F32, tag="consts")
            nc.sync.dma_start(consts[:], CONSTS[:, :])
            w1b_st = consts[:, 0:1]
            w2b_st = consts[:, 1:2]
            vb_bc = consts[:, 2:3]

            vv_f32 = cpool.tile([128, 512], F32, tag="vv_f32")
            nc.sync.dma_start(vv_f32[:], VVARS[:, :])

            # projections: q1T_stack / k2T_stack  [128=(d,bh), 512]
            # k2t is bf16 so the per-q broadcast-add runs in the DVE packed
            # (2x) perf mode; q1t stays f32 (scalar operand is exempt).
            q1t = cpool.tile([128, SQ], F32, tag="q1t")
            k2t = cpool.tile([128, SK], mybir.dt.bfloat16, tag="k2t")
            ps_q1 = pspool.tile([128, SQ], F32, tag="setup")
            nc.tensor.matmul(ps_q1[:], w1tb[:], qts[:], start=True, stop=True)
            nc.vector.tensor_scalar_add(q1t[:], ps_q1[:], w1b_st)
            ps_k2 = pspool.tile([128, SK], F32, tag="setup")
            nc.tensor.matmul(ps_k2[:], w2tb[:], kts[:], start=True, stop=True)
            # k2 bias-add on the (head-idle) ACT engine, off the DVE chain
            # that feeds the first stage adds; identity shares tanh's table
            nc.scalar.activation(
                k2t[:], ps_k2[:], mybir.ActivationFunctionType.Identity,
                bias=w2b_st,
            )

            vvars = cpool.tile([128, 512], mm_dt, tag="vvars")

            # ---------------- main loop ----------------
            qcur = 0
            ps_e = None
            vvars_done = False
            for bsz in BATCHES:
                stage = spool.tile(
                    [128, MAXB * SK], mybir.dt.bfloat16, tag="stage"
                )
                for j in range(bsz):
                    q = qcur + j
                    nc.vector.tensor_scalar_add(
                        stage[:, j * SK : (j + 1) * SK],
                        k2t[:],
                        q1t[:, q : q + 1],
                    )
                if not vvars_done:
                    # round the host-built V variants to the matmul dtype;
                    # emitted after the first adds so the DVE reaches the
                    # first stage batch sooner (only the first matmul needs it)
                    nc.vector.tensor_copy(vvars[:], vv_f32[:])
                    vvars_done = True
                th = tpool.tile([128, MAXB * SK], mm_dt, tag="tanh")
                nc.scalar.activation(
                    th[:, 0 : bsz * SK],
                    stage[:, 0 : bsz * SK],
                    mybir.ActivationFunctionType.Tanh,
                )
                for j in range(bsz):
                    q = qcur + j
                    t = q % QPT  # variant within psum tile, 0..15
                    if t == 0:
                        ps_e = ppool.tile([32, 512], F32, tag="ps_e")
                    nc.tensor.matmul(
                        ps_e[:],
                        vvars[:, 32 * t : 32 * t + 32],
                        th[:, j * SK : (j + 1) * SK],
                        start=(t == 0),
                        stop=(t == QPT - 1),
                    )
                    if t == QPT - 1:
                        # rows p = 2*(q%16) + bh -> out row 32g + p = 2q + bh
                        g = q // QPT
                        ev = epool.tile([32, 512], F32, tag="ev")
                        nc.vector.tensor_scalar_add(
                            ev[:], ps_e[:], vb_bc[0:32, 0:1]
                        )
                        nc.sync.dma_start(out[32 * g : 32 * g + 32, :], ev[:])
                qcur += bsz

    nc.compile()
    return nc


_NC_CACHE = None
LAST_RESULTS = None


def _get_nc():
    global _NC_CACHE
    if _NC_CACHE is None:
        _NC_CACHE = build_nc()
    return _NC_CACHE


def make_in_maps(Q, K, W1_w, W1_b, W2_w, W2_b, V_w, V_b):
    """Host-side marshalling: shard (b,h) across cores, transpose layouts,
    prebuild the block-diagonal weights, constant columns and V variants."""
    import ml_dtypes

    f = np.float32
    bf = ml_dtypes.bfloat16
    Qf = np.ascontiguousarray(Q, dtype=f).reshape(BH, SQ, D)
    Kf = np.ascontiguousarray(K, dtype=f).reshape(BH, SK, D)
    W1T = np.asarray(W1_w, dtype=f).T  # [d, e]
    W2T = np.asarray(W2_w, dtype=f).T

    w1tb = np.zeros((128, 128), dtype=bf)
    w1tb[0:64, 0:64] = W1T.astype(bf)
    w1tb[64:128, 64:128] = W1T.astype(bf)
    w2tb = np.zeros((128, 128), dtype=bf)
    w2tb[0:64, 0:64] = W2T.astype(bf)
    w2tb[64:128, 64:128] = W2T.astype(bf)

    consts = np.zeros((128, 3), dtype=f)
    consts[0:64, 0] = consts[64:128, 0] = np.asarray(W1_b, dtype=f).ravel()
    consts[0:64, 1] = consts[64:128, 1] = np.asarray(W2_b, dtype=f).ravel()
    consts[:, 2] = np.asarray(V_b, dtype=f).ravel()[0]

    vvars = np.zeros((128, 512), dtype=f)
    vw = np.asarray(V_w, dtype=f).ravel()  # [64]
    for t in range(16):
        vvars[0:64, 32 * t + 2 * t] = vw
        vvars[64:128, 32 * t + 2 * t + 1] = vw

    in_maps = []
    for c in range(NCORES):
        sl = slice(c * BH_PER_CORE, (c + 1) * BH_PER_CORE)
        qts = np.ascontiguousarray(
            Qf[sl].transpose(0, 2, 1).reshape(128, SQ).astype(bf)
        )  # [2*64, SQ]
        kts = np.ascontiguousarray(
            Kf[sl].transpose(0, 2, 1).reshape(128, SK).astype(bf)
        )
        in_maps.append(
            {
                "QTS": qts,
                "KTS": kts,
                "W1TB": w1tb,
                "W2TB": w2tb,
                "CONSTS": consts,
                "VVARS": vvars,
            }
        )
    return in_maps


def kernel(**inputs) -> np.ndarray:
    global LAST_RESULTS
    from concourse.bass_utils import run_bass_kernel_spmd

    nc = _get_nc()
    in_maps = make_in_maps(**inputs)
    try:
        res = run_bass_kernel_spmd(nc, in_maps, core_ids=list(range(NCORES)))
    except Exception:
        # transient NRT device errors have been observed; retry once
        res = run_bass_kernel_spmd(nc, in_maps, core_ids=list(range(NCORES)))
    LAST_RESULTS = res
    per_core = [
        r["out"].reshape(SQ, BH_PER_CORE, SK).transpose(1, 0, 2) for r in res.results
    ]
    full = np.concatenate(per_core, axis=0)  # [16, 512, 512]
    return np.ascontiguousarray(full.reshape(B, H, SQ, SK), dtype=np.float32)
